# revision 6
# baseline (speedup 1.0000x reference)
"""Trainium2 Bass kernel for a DynamicConv decoder layer.

Computation (fairseq DynamicConvDecoderLayer, eval mode, normalize_after):
    h  = x @ w1.T + b1                       # [T,B,E] -> [T,B,C]
    w  = softmax((h @ ww.T + bw) per-head)   # dynamic conv weights [T,B,H,K]
    c  = causal banded aggregation of h with per-position weights
    h2 = c @ w2.T + b2
    out = LayerNorm(x + h2) * gamma + beta

Distribution: data-parallel over batch (B=16 -> 2 per core on 8 cores).

Per-core algorithm (tokens laid out b-major, m = b*T + t):
  - Phase A: h1 = x @ w1.T (token-partition layout) via fp32r matmuls,
    lhsT = x^T (host pre-transposed), rhs = w1^T.
  - Phase B: conv logits computed directly from x with the host-fused
    weight (ww @ w1)^T, so h1 is never needed in C-partition layout.
  - Softmax per (token, head) on DVE/ACT; result cast to bf16.
  - Band build: GPSIMD local_scatter skews the per-token weight rows into
    an aligned band block Band[tau_out, tau_src] (per head), then PE
    transposes 128x128 chunks to get Band^T[tau_src, tau_out] in SBUF.
  - Conv: per (head, tau_out tile) 2 accumulating bf16 matmuls:
    conv^T[r, tau_out] = sum_{tau_src} h1[tau_src, r] * Band^T[tau_src, tau_out].
    Output lands directly in C-partition layout for phase D.
  - Phase D: h2 = conv @ w2.T with lhsT = conv^T; residual x and biases are
    added on the PE itself (identity / rank-1 matmuls into the same PSUM).
  - LayerNorm on DVE/ACT; rstd = exp(-0.5*ln(var+eps)) keeps ACT on a
    single table set (natural log + exp) to avoid table thrash.
"""

import sys
import os

sys.path.insert(0, "/opt/trn_rl_repo")

import numpy as np
from contextlib import ExitStack

import concourse.bass as bass
import concourse.bacc as bacc
import concourse.mybir as mybir
from concourse import tile

T, B, E = 2048, 16, 1024
CDIM, H, KW = 1024, 16, 31
R = CDIM // H            # 64 channels per head
NB = 2                   # batch shard per core
NCORES = 8
P = 128
EPS = 1e-5

AF = mybir.ActivationFunctionType
ALU = mybir.AluOpType

# local_scatter groups: (head0, nheads); num_idxs = nh*31 must be even,
# num_elems = nh*256 must be < 2048.
SCAT_GROUPS = [(0, 6), (6, 6), (12, 4)]


def _build(t_loc: int, trivial_affine: bool) -> bacc.Bacc:
    f32 = mybir.dt.float32
    f32r = mybir.dt.float32r
    bf16 = mybir.dt.bfloat16
    i16 = mybir.dt.int16

    m_loc = NB * t_loc           # tokens per core
    nt = m_loc // P              # token tiles
    tpb = t_loc // P             # tiles per local batch
    nblk = max(m_loc // 512, 1)  # 512-token xT blocks
    tpblk = nt // nblk           # tiles per block (4)

    nc = bacc.Bacc()

    xT_d = nc.dram_tensor("xT", [E, m_loc], f32r, kind="ExternalInput")
    xtok_d = nc.dram_tensor("xtok", [m_loc, E], f32r, kind="ExternalInput")
    w1T_d = nc.dram_tensor("w1T", [E, CDIM], f32r, kind="ExternalInput")
    wfT_d = nc.dram_tensor("wfT", [E, H * KW], f32r, kind="ExternalInput")
    w2T_d = nc.dram_tensor("w2T", [CDIM, E], f32r, kind="ExternalInput")
    b1r_d = nc.dram_tensor("b1r", [1, CDIM], f32r, kind="ExternalInput")
    bwr_d = nc.dram_tensor("bwr", [1, H * KW], f32r, kind="ExternalInput")
    b2r_d = nc.dram_tensor("b2r", [1, E], f32r, kind="ExternalInput")
    ones_d = nc.dram_tensor("ones", [1, P], f32r, kind="ExternalInput")
    identb_d = nc.dram_tensor("identb", [P, P], bf16, kind="ExternalInput")
    identr_d = nc.dram_tensor("identr", [P, P], f32r, kind="ExternalInput")
    idx_d = [
        nc.dram_tensor(f"idx{g}", [P, nh * KW], i16, kind="ExternalInput")
        for g, (_, nh) in enumerate(SCAT_GROUPS)
    ]
    if not trivial_affine:
        gam_d = nc.dram_tensor("gamma_bc", [P, E], f32, kind="ExternalInput")
        bet_d = nc.dram_tensor("beta_bc", [P, E], f32, kind="ExternalInput")
    out_d = nc.dram_tensor("out", [m_loc, E], f32, kind="ExternalOutput")

    with tile.TileContext(nc) as tc, ExitStack() as ctx:
        const = ctx.enter_context(tc.tile_pool(name="const", bufs=1))
        xt_p = ctx.enter_context(tc.tile_pool(name="xt", bufs=2))
        xtk_p = ctx.enter_context(tc.tile_pool(name="xtk", bufs=2))
        h1_p = ctx.enter_context(tc.tile_pool(name="h1", bufs=4))
        sm_p = ctx.enter_context(tc.tile_pool(name="sm", bufs=2))
        bu_p = ctx.enter_context(tc.tile_pool(name="bu", bufs=2))
        bt_p = ctx.enter_context(tc.tile_pool(name="bt", bufs=32))
        ct_p = ctx.enter_context(tc.tile_pool(name="ct", bufs=8))
        z_p = ctx.enter_context(tc.tile_pool(name="z", bufs=2))
        out_p = ctx.enter_context(tc.tile_pool(name="outp", bufs=2))
        ps_ab = ctx.enter_context(tc.tile_pool(name="psab", bufs=2, space="PSUM"))
        ps_d = ctx.enter_context(tc.tile_pool(name="psd", bufs=2, space="PSUM"))
        ps_t = ctx.enter_context(tc.tile_pool(name="pst", bufs=2, space="PSUM"))
        ps_c = ctx.enter_context(tc.tile_pool(name="psc", bufs=2, space="PSUM"))

        # resident constants
        w1T = [const.tile([P, CDIM], f32r, tag=f"w1T{e}", name=f"w1T{e}") for e in range(8)]
        wfT = [const.tile([P, H * KW], f32r, tag=f"wfT{e}", name=f"wfT{e}") for e in range(8)]
        w2T = [const.tile([P, E], f32r, tag=f"w2T{c}", name=f"w2T{c}") for c in range(8)]
        for e in range(8):
            nc.sync.dma_start(w1T[e][:], w1T_d[e * P:(e + 1) * P, :])
            nc.sync.dma_start(wfT[e][:], wfT_d[e * P:(e + 1) * P, :])
            nc.sync.dma_start(w2T[e][:], w2T_d[e * P:(e + 1) * P, :])
        b1r = const.tile([1, CDIM], f32r, tag="b1r")
        bwr = const.tile([1, H * KW], f32r, tag="bwr")
        b2r = const.tile([1, E], f32r, tag="b2r")
        ones = const.tile([1, P], f32r, tag="ones")
        identb = const.tile([P, P], bf16, tag="identb")
        identr = const.tile([P, P], f32r, tag="identr")
        nc.sync.dma_start(b1r[:], b1r_d[:])
        nc.sync.dma_start(bwr[:], bwr_d[:])
        nc.sync.dma_start(b2r[:], b2r_d[:])
        nc.sync.dma_start(ones[:], ones_d[:])
        nc.sync.dma_start(identb[:], identb_d[:])
        nc.sync.dma_start(identr[:], identr_d[:])
        eps_t = const.tile([P, 1], f32, tag="eps")
        nc.vector.memset(eps_t[:], EPS)
        idx_t = []
        for g, (_, nh) in enumerate(SCAT_GROUPS):
            it = const.tile([P, nh * KW], i16, tag=f"idx{g}", name=f"idxt{g}")
            nc.sync.dma_start(it[:], idx_d[g][:])
            idx_t.append(it)
        if not trivial_affine:
            gam_t = const.tile([P, E], f32, tag="gam")
            bet_t = const.tile([P, E], f32, tag="bet")
            nc.sync.dma_start(gam_t[:], gam_d[:])
            nc.sync.dma_start(bet_t[:], bet_d[:])

        xt = None
        h1_prev = None

        for i in range(nt):
            i_b = i % tpb
            j = i % tpblk
            if j == 0:
                blk = i // tpblk
                bw_ = min(512, m_loc)
                xt = [xt_p.tile([P, bw_], f32r, tag=f"xt{e}", name=f"xtt{e}") for e in range(8)]
                for e in range(8):
                    nc.sync.dma_start(
                        xt[e][:], xT_d[e * P:(e + 1) * P, blk * bw_:(blk + 1) * bw_]
                    )
            js = slice(j * P, (j + 1) * P)

            # ---- Phase A: h1 tile [128 tok, CDIM] bf16 ----
            h1_t = h1_p.tile([P, CDIM], bf16, tag="h1")
            for cb in range(2):
                pa = ps_ab.tile([P, 512], f32, tag="psab")
                for e in range(8):
                    nc.tensor.matmul(
                        pa[:], xt[e][:, js], w1T[e][:, cb * 512:(cb + 1) * 512],
                        start=(e == 0), stop=False,
                    )
                nc.tensor.matmul(
                    pa[:], ones[:], b1r[:, cb * 512:(cb + 1) * 512],
                    start=False, stop=True,
                )
                nc.scalar.copy(h1_t[:, cb * 512:(cb + 1) * 512], pa[:])

            # ---- Phase B: conv logits + softmax -> bf16 weights ----
            pb = ps_ab.tile([P, H * KW], f32, tag="psab")
            for e in range(8):
                nc.tensor.matmul(
                    pb[:], xt[e][:, js], wfT[e][:], start=(e == 0), stop=False
                )
            nc.tensor.matmul(pb[:], ones[:], bwr[:], start=False, stop=True)
            expw = sm_p.tile([P, H * KW], f32, tag="expw")
            nc.scalar.activation(expw[:], pb[:], AF.Exp)
            sums = sm_p.tile([P, H], f32, tag="sums")
            nc.vector.tensor_reduce(
                sums[:], expw[:].rearrange("p (h k) -> p h k", k=KW),
                axis=mybir.AxisListType.X, op=ALU.add,
            )
            rsum = sm_p.tile([P, H], f32, tag="rsum")
            nc.vector.reciprocal(rsum[:], sums[:])
            wbf = sm_p.tile([P, H * KW], bf16, tag="wbf")
            for h in range(H):
                nc.vector.tensor_scalar_mul(
                    wbf[:, h * KW:(h + 1) * KW],
                    expw[:, h * KW:(h + 1) * KW],
                    rsum[:, h:h + 1],
                )

            # ---- band build: scatter to Band[tau_out, (h, sigma)] ----
            bandu = bu_p.tile([P, H * 256], bf16, tag="bandu")
            for g, (h0, nh) in enumerate(SCAT_GROUPS):
                nc.gpsimd.local_scatter(
                    bandu[:, h0 * 256:(h0 + nh) * 256],
                    wbf[:, h0 * KW:(h0 + nh) * KW],
                    idx_t[g][:],
                    channels=P, num_elems=nh * 256, num_idxs=nh * KW,
                )

            # ---- PE transposes: Band^T[tau_src, tau_out] per head ----
            bt_lo = [None] * H
            bt_hi = [None] * H
            for h in range(H):
                for half in range(2):
                    if half == 0 and i_b == 0:
                        continue
                    pt = ps_t.tile([P, P], bf16, tag="pst")
                    nc.tensor.transpose(
                        pt[:],
                        bandu[:, h * 256 + half * P: h * 256 + (half + 1) * P],
                        identb[:],
                    )
                    bt = bt_p.tile([P, P], bf16, tag="bt")
                    if (h + half) % 2 == 0:
                        nc.scalar.copy(bt[:], pt[:])
                    else:
                        nc.vector.tensor_copy(bt[:], pt[:])
                    if half == 0:
                        bt_lo[h] = bt
                    else:
                        bt_hi[h] = bt

            # ---- conv matmuls: conv^T in C-partition layout ----
            ct_tiles = []
            for hp in range(8):
                pc = ps_c.tile([P, P], f32, tag="psc")
                for hh in range(2):
                    h = hp * 2 + hh
                    ms = slice(hh * 64, hh * 64 + 64)
                    first = True
                    if i_b > 0:
                        nc.tensor.matmul(
                            pc[ms, :], h1_prev[:, h * R:(h + 1) * R], bt_lo[h][:],
                            start=True, stop=False,
                        )
                        first = False
                    nc.tensor.matmul(
                        pc[ms, :], h1_t[:, h * R:(h + 1) * R], bt_hi[h][:],
                        start=first, stop=True,
                    )
                ct = ct_p.tile([P, P], f32r, tag="ct")
                if hp % 2 == 0:
                    nc.scalar.copy(ct[:], pc[:])
                else:
                    nc.vector.tensor_copy(ct[:], pc[:])
                ct_tiles.append(ct)

            # ---- Phase D: h2 + x + b2, then LayerNorm ----
            xtok_t = xtk_p.tile([P, E], f32r, tag="xtok")
            nc.sync.dma_start(xtok_t[:], xtok_d[i * P:(i + 1) * P, :])
            zsb = z_p.tile([P, E], f32, tag="zsb")
            for eb in range(2):
                pd = ps_d.tile([P, 512], f32, tag="psd")
                for hp in range(8):
                    nc.tensor.matmul(
                        pd[:], ct_tiles[hp][:],
                        w2T[hp][:, eb * 512:(eb + 1) * 512],
                        start=(hp == 0), stop=False,
                    )
                nc.tensor.matmul(
                    pd[:], ones[:], b2r[:, eb * 512:(eb + 1) * 512],
                    start=False, stop=False,
                )
                nc.tensor.matmul(
                    pd[:], identr[:], xtok_t[:, eb * 512:(eb + 1) * 512],
                    start=False, stop=True,
                )
                nc.scalar.copy(zsb[:, eb * 512:(eb + 1) * 512], pd[:])

            # LayerNorm stats
            st = sm_p.tile([P, 8], f32, tag="st")
            nc.vector.reduce_sum(
                st[:, 0:1], zsb[:, 0:512], axis=mybir.AxisListType.X
            )
            nc.vector.reduce_sum(
                st[:, 1:2], zsb[:, 512:1024], axis=mybir.AxisListType.X
            )
            nc.vector.tensor_reduce(
                st[:, 2:3], st[:, 0:2], axis=mybir.AxisListType.X, op=ALU.add
            )
            nc.vector.tensor_scalar_mul(st[:, 3:4], st[:, 2:3], -1.0 / E)  # negmean
            for eb in range(2):
                sq = z_p.tile([P, 512], f32, tag="sq")
                nc.vector.scalar_tensor_tensor(
                    sq[:], zsb[:, eb * 512:(eb + 1) * 512], 0.0,
                    zsb[:, eb * 512:(eb + 1) * 512],
                    op0=ALU.add, op1=ALU.mult, accum_out=st[:, 4 + eb:5 + eb],
                )
            nc.vector.tensor_reduce(
                st[:, 6:7], st[:, 4:6], axis=mybir.AxisListType.X, op=ALU.add
            )
            # m2 = negmean^2
            nc.vector.tensor_scalar(
                st[:, 7:8], st[:, 3:4], st[:, 3:4], None, op0=ALU.mult
            )
            # var = sumsq/E - m2  (written over st[:,6:7])
            nc.vector.tensor_scalar(
                st[:, 6:7], st[:, 6:7], 1.0 / E, st[:, 7:8],
                op0=ALU.mult, op1=ALU.subtract,
            )
            # rstd = exp(-0.5 * ln(var + eps))
            lnv = sm_p.tile([P, 2], f32, tag="lnv")
            nc.scalar.activation(lnv[:, 0:1], st[:, 6:7], AF.Ln, bias=eps_t[:, 0:1])
            nc.scalar.activation(lnv[:, 1:2], lnv[:, 0:1], AF.Exp, scale=-0.5)

            out_t = out_p.tile([P, E], f32, tag="outt")
            for eb in range(2):
                nc.vector.tensor_scalar(
                    out_t[:, eb * 512:(eb + 1) * 512],
                    zsb[:, eb * 512:(eb + 1) * 512],
                    st[:, 3:4], lnv[:, 1:2],
                    op0=ALU.add, op1=ALU.mult,
                )
            if not trivial_affine:
                nc.vector.tensor_mul(out_t[:], out_t[:], gam_t[:])
                nc.vector.tensor_add(out_t[:], out_t[:], bet_t[:])
            nc.sync.dma_start(out_d[i * P:(i + 1) * P, :], out_t[:])

            h1_prev = h1_t

    nc.finalize()
    return nc


def _scatter_idx() -> list[np.ndarray]:
    tables = []
    for h0, nh in SCAT_GROUPS:
        t = np.zeros((P, nh * KW), np.int16)
        for p in range(P):
            for hl in range(nh):
                for k in range(KW):
                    t[p, hl * KW + k] = hl * 256 + p + k + 98
        tables.append(t)
    return tables


_CACHE: dict = {}


def _get_nc(t_loc: int, trivial: bool):
    key = (t_loc, trivial)
    if key not in _CACHE:
        _CACHE[key] = _build(t_loc, trivial)
    return _CACHE[key]


def kernel(x, w1, b1, ww, bw, w2, b2, gamma, beta):
    x = np.asarray(x, np.float32)
    w1 = np.asarray(w1, np.float32)
    b1 = np.asarray(b1, np.float32)
    ww = np.asarray(ww, np.float32)
    bw = np.asarray(bw, np.float32)
    w2 = np.asarray(w2, np.float32)
    b2 = np.asarray(b2, np.float32)
    gamma = np.asarray(gamma, np.float32)
    beta = np.asarray(beta, np.float32)

    t_loc, b_full, e = x.shape
    assert e == E and b_full == B

    trivial = bool(np.all(gamma == 1.0) and np.all(beta == 0.0))
    nc = _get_nc(t_loc, trivial)

    wf = (ww.astype(np.float64) @ w1.astype(np.float64)).astype(np.float32)
    bwf = (ww.astype(np.float64) @ b1.astype(np.float64)).astype(np.float32) + bw

    bf16 = mybir.dt.np(mybir.dt.bfloat16)
    common = {
        "w1T": np.ascontiguousarray(w1.T),
        "wfT": np.ascontiguousarray(wf.T),
        "w2T": np.ascontiguousarray(w2.T),
        "b1r": b1[None, :],
        "bwr": bwf[None, :],
        "b2r": b2[None, :],
        "ones": np.ones((1, P), np.float32),
        "identb": np.eye(P).astype(bf16),
        "identr": np.eye(P, dtype=np.float32),
    }
    for g, t in enumerate(_scatter_idx()):
        common[f"idx{g}"] = t
    if not trivial:
        common["gamma_bc"] = np.broadcast_to(gamma, (P, E)).copy()
        common["beta_bc"] = np.broadcast_to(beta, (P, E)).copy()

    in_maps = []
    for c in range(NCORES):
        xs = x[:, NB * c:NB * (c + 1), :]                       # [T, 2, E]
        xtok = np.ascontiguousarray(xs.transpose(1, 0, 2)).reshape(NB * t_loc, E)
        xT = np.ascontiguousarray(xs.transpose(2, 1, 0)).reshape(E, NB * t_loc)
        m = dict(common)
        m["xT"] = xT
        m["xtok"] = np.ascontiguousarray(xtok)
        in_maps.append(m)

    from concourse.bass_utils import run_bass_kernel_spmd

    res = run_bass_kernel_spmd(nc, in_maps, core_ids=list(range(NCORES)))

    out = np.empty((t_loc, B, E), np.float32)
    for c in range(NCORES):
        oc = res.results[c]["out"].reshape(NB, t_loc, E)
        for bl in range(NB):
            out[:, NB * c + bl, :] = oc[bl]
    return out


# revision 10
# speedup vs baseline: 1.0913x; 1.0913x over previous
"""Trainium2 Bass kernel for a DynamicConv decoder layer.

Computation (fairseq DynamicConvDecoderLayer, eval mode, normalize_after):
    h  = x @ w1.T + b1                       # [T,B,E] -> [T,B,C]
    w  = softmax((h @ ww.T + bw) per-head)   # dynamic conv weights [T,B,H,K]
    c  = causal banded aggregation of h with per-position weights
    h2 = c @ w2.T + b2
    out = LayerNorm(x + h2) * gamma + beta

Distribution: data-parallel over batch (B=16 -> 2 per core on 8 cores).

Per-core algorithm (tokens laid out b-major, m = b*T + t):
  - Phase A: h1 = x @ w1.T (token-partition layout) via fp32r matmuls,
    lhsT = x^T (host pre-transposed), rhs = w1^T.
  - Phase B: conv logits computed directly from x with the host-fused
    weight (ww @ w1)^T, so h1 is never needed in C-partition layout.
  - Softmax per (token, head) on DVE/ACT; result cast to bf16.
  - Band build: GPSIMD local_scatter skews the per-token weight rows into
    an aligned band block Band[tau_out, tau_src] (per head), then PE
    transposes 128x128 chunks (4 per PSUM bank) to Band^T[tau_src, tau_out].
  - Conv: per (head, tau_out tile) 2 accumulating bf16 matmuls:
    conv^T[r, tau_out] = sum_{tau_src} h1[tau_src, r] * Band^T[tau_src, tau_out],
    4 head-pairs packed per PSUM bank; output lands in C-partition layout.
  - Phase D: h2 = conv @ w2.T with lhsT = conv^T; residual + sum(z) ride the
    PSUM->SBUF evacuation (scalar_tensor_tensor with accum_out); sum(z^2)
    rides an ACT Square pass.
  - LayerNorm rstd = exp(-0.5*ln(var+eps)); all ACT functions (Exp, Ln,
    Copy, Square) live in the single `natural_log_exp_and_others` table set.
"""

import sys
import os

sys.path.insert(0, "/opt/trn_rl_repo")

import numpy as np
from contextlib import ExitStack

import concourse.bass as bass
import concourse.bacc as bacc
import concourse.mybir as mybir
from concourse import tile

T, B, E = 2048, 16, 1024
CDIM, H, KW = 1024, 16, 31
R = CDIM // H            # 64 channels per head
NB = 2                   # batch shard per core
NCORES = 8
P = 128
EPS = 1e-5

AF = mybir.ActivationFunctionType
ALU = mybir.AluOpType

# local_scatter groups: (head0, nheads); num_idxs = nh*31 must be even,
# num_elems = nh*256 must be < 2048.
SCAT_GROUPS = [(0, 6), (6, 6), (12, 4)]

_ONE_TABLE = "natural_log_exp_and_others"


class _Bacc(bacc.Bacc):
    """Bacc with the ACT table list restricted to one set covering every
    activation function this kernel uses (Exp, Ln, Copy, Square, Identity)
    — the default per-activation selection ping-pongs between sets,
    costing a ~1.3us table load per switch."""

    def insert_act_table_loads(self):
        from concourse.hw_specs import get_activation_tables
        import bass_rust as _bass_rust

        has_activation = any(
            isinstance(i, mybir.InstActivation)
            for b in self.main_func.blocks
            for i in b.instructions
        )
        if not has_activation:
            return
        # Keep every entry (act_func_set_id is positional into
        # act_info.json) but empty the other sets so the selector can
        # only ever pick _ONE_TABLE.
        if os.environ.get("V2_TBL", "1") == "1":
            tables = [
                (k, v if k == _ONE_TABLE else set())
                for k, v in get_activation_tables(self.m.arch).items()
            ]
        else:
            tables = list(get_activation_tables(self.m.arch).items())
        assert any(v for _, v in tables)
        import bass_rust
        bass_rust.insert_act_table_loads(self, tables)


def _build(t_loc: int, trivial_affine: bool, trivial_bias: bool) -> bacc.Bacc:
    f32 = mybir.dt.float32
    f32r = mybir.dt.float32r
    bf16 = mybir.dt.bfloat16
    i16 = mybir.dt.int16

    m_loc = NB * t_loc           # tokens per core
    nt = m_loc // P              # token tiles
    tpb = t_loc // P             # tiles per local batch
    nblk = max(m_loc // 512, 1)  # 512-token xT blocks
    tpblk = nt // nblk           # tiles per block (4)

    nc = _Bacc()

    xT_d = nc.dram_tensor("xT", [E, m_loc], f32r, kind="ExternalInput")
    xtok_d = nc.dram_tensor("xtok", [m_loc, E], f32, kind="ExternalInput")
    w1T_d = nc.dram_tensor("w1T", [E, CDIM], f32r, kind="ExternalInput")
    wfT_d = nc.dram_tensor("wfT", [E, H * KW], f32r, kind="ExternalInput")
    w2T_d = nc.dram_tensor("w2T", [CDIM, E], f32r, kind="ExternalInput")
    identb_d = nc.dram_tensor("identb", [P, P], bf16, kind="ExternalInput")
    idx_d = [
        nc.dram_tensor(f"idx{g}", [P, nh * KW], i16, kind="ExternalInput")
        for g, (_, nh) in enumerate(SCAT_GROUPS)
    ]
    if not trivial_bias:
        b1r_d = nc.dram_tensor("b1r", [1, CDIM], f32r, kind="ExternalInput")
        bwr_d = nc.dram_tensor("bwr", [1, H * KW], f32r, kind="ExternalInput")
        b2r_d = nc.dram_tensor("b2r", [1, E], f32r, kind="ExternalInput")
        ones_d = nc.dram_tensor("ones", [1, P], f32r, kind="ExternalInput")
    if not trivial_affine:
        gam_d = nc.dram_tensor("gamma_bc", [P, E], f32, kind="ExternalInput")
        bet_d = nc.dram_tensor("beta_bc", [P, E], f32, kind="ExternalInput")
    out_d = nc.dram_tensor("out", [m_loc, E], f32, kind="ExternalOutput")

    with tile.TileContext(nc) as tc, ExitStack() as ctx:
        const = ctx.enter_context(tc.tile_pool(name="const", bufs=1))
        xt_p = ctx.enter_context(tc.tile_pool(name="xt", bufs=2))
        xtk_p = ctx.enter_context(tc.tile_pool(name="xtk", bufs=2))
        h1_p = ctx.enter_context(tc.tile_pool(name="h1", bufs=4))
        sm_p = ctx.enter_context(tc.tile_pool(name="sm", bufs=2))
        bu_p = ctx.enter_context(tc.tile_pool(name="bu", bufs=2))
        bt_p = ctx.enter_context(tc.tile_pool(name="bt", bufs=12))
        ct_p = ctx.enter_context(tc.tile_pool(name="ct", bufs=3))
        z_p = ctx.enter_context(tc.tile_pool(name="z", bufs=2))
        out_p = ctx.enter_context(tc.tile_pool(name="outp", bufs=2))
        ps_ab = ctx.enter_context(tc.tile_pool(name="psab", bufs=2, space="PSUM"))
        ps_d = ctx.enter_context(tc.tile_pool(name="psd", bufs=2, space="PSUM"))
        ps_t = ctx.enter_context(tc.tile_pool(name="pst", bufs=2, space="PSUM"))
        ps_c = ctx.enter_context(tc.tile_pool(name="psc", bufs=2, space="PSUM"))

        # resident constants
        w1T = [const.tile([P, CDIM], f32r, tag=f"w1T{e}", name=f"w1T{e}")
               for e in range(8)]
        wfT = [const.tile([P, H * KW], f32r, tag=f"wfT{e}", name=f"wfT{e}")
               for e in range(8)]
        w2T = [const.tile([P, E], f32r, tag=f"w2T{c}", name=f"w2T{c}")
               for c in range(8)]
        for e in range(8):
            nc.sync.dma_start(w1T[e][:], w1T_d[e * P:(e + 1) * P, :])
            nc.sync.dma_start(wfT[e][:], wfT_d[e * P:(e + 1) * P, :])
            nc.sync.dma_start(w2T[e][:], w2T_d[e * P:(e + 1) * P, :])
        identb = const.tile([P, P], bf16, tag="identb")
        nc.sync.dma_start(identb[:], identb_d[:])
        eps_t = const.tile([P, 1], f32, tag="eps")
        nc.vector.memset(eps_t[:], EPS)
        if not trivial_bias:
            b1r = const.tile([1, CDIM], f32r, tag="b1r")
            bwr = const.tile([1, H * KW], f32r, tag="bwr")
            b2r = const.tile([1, E], f32r, tag="b2r")
            ones = const.tile([1, P], f32r, tag="ones")
            nc.sync.dma_start(b1r[:], b1r_d[:])
            nc.sync.dma_start(bwr[:], bwr_d[:])
            nc.sync.dma_start(b2r[:], b2r_d[:])
            nc.sync.dma_start(ones[:], ones_d[:])
        idx_t = []
        for g, (_, nh) in enumerate(SCAT_GROUPS):
            it = const.tile([P, nh * KW], i16, tag=f"idx{g}", name=f"idxt{g}")
            nc.sync.dma_start(it[:], idx_d[g][:])
            idx_t.append(it)
        if not trivial_affine:
            gam_t = const.tile([P, E], f32, tag="gam")
            bet_t = const.tile([P, E], f32, tag="bet")
            nc.sync.dma_start(gam_t[:], gam_d[:])
            nc.sync.dma_start(bet_t[:], bet_d[:])

        xt = None
        h1_prev = None

        for i in range(nt):
            i_b = i % tpb
            j = i % tpblk
            if j == 0:
                blk = i // tpblk
                bw_ = min(512, m_loc)
                xt = [xt_p.tile([P, bw_], f32r, tag=f"xt{e}", name=f"xtt{e}")
                      for e in range(8)]
                for e in range(8):
                    nc.sync.dma_start(
                        xt[e][:], xT_d[e * P:(e + 1) * P, blk * bw_:(blk + 1) * bw_]
                    )
            js = slice(j * P, (j + 1) * P)

            # ---- Phase A: h1 tile [128 tok, CDIM] bf16 ----
            h1_t = h1_p.tile([P, CDIM], bf16, tag="h1")
            for cb in range(2):
                pa = ps_ab.tile([P, 512], f32, tag="psab")
                for e in range(8):
                    nc.tensor.matmul(
                        pa[:], xt[e][:, js], w1T[e][:, cb * 512:(cb + 1) * 512],
                        start=(e == 0), stop=(e == 7 and trivial_bias),
                    )
                if not trivial_bias:
                    nc.tensor.matmul(
                        pa[:], ones[:], b1r[:, cb * 512:(cb + 1) * 512],
                        start=False, stop=True,
                    )
                nc.scalar.copy(h1_t[:, cb * 512:(cb + 1) * 512], pa[:])

            # ---- Phase B: conv logits + softmax -> bf16 weights ----
            pb = ps_ab.tile([P, H * KW], f32, tag="psab")
            for e in range(8):
                nc.tensor.matmul(
                    pb[:], xt[e][:, js], wfT[e][:],
                    start=(e == 0), stop=(e == 7 and trivial_bias),
                )
            if not trivial_bias:
                nc.tensor.matmul(pb[:], ones[:], bwr[:], start=False, stop=True)
            expw = sm_p.tile([P, H * KW], f32, tag="expw")
            nc.scalar.activation(expw[:], pb[:], AF.Exp)
            sums = sm_p.tile([P, H], f32, tag="sums")
            nc.vector.tensor_reduce(
                sums[:], expw[:].rearrange("p (h k) -> p h k", k=KW),
                axis=mybir.AxisListType.X, op=ALU.add,
            )
            rsum = sm_p.tile([P, H], f32, tag="rsum")
            nc.vector.reciprocal(rsum[:], sums[:])
            wbf = sm_p.tile([P, H * KW], bf16, tag="wbf")
            for h in range(H):
                nc.vector.tensor_scalar_mul(
                    wbf[:, h * KW:(h + 1) * KW],
                    expw[:, h * KW:(h + 1) * KW],
                    rsum[:, h:h + 1],
                )

            # ---- band build: scatter to Band[tau_out, (h, sigma)] ----
            bandu = bu_p.tile([P, H * 256], bf16, tag="bandu")
            for g, (h0, nh) in enumerate(SCAT_GROUPS):
                nc.gpsimd.local_scatter(
                    bandu[:, h0 * 256:(h0 + nh) * 256],
                    wbf[:, h0 * KW:(h0 + nh) * KW],
                    idx_t[g][:],
                    channels=P, num_elems=nh * 256, num_idxs=nh * KW,
                )

            # ---- PE transposes: Band^T[tau_src, tau_out], 4 chunks/bank ----
            # i_b>0: group g covers head pair (2g, 2g+1): [lo0|hi0|lo1|hi1]
            # i_b==0: group g covers heads 4g..4g+3: [hi|hi|hi|hi]
            ngrp = 8 if i_b > 0 else 4
            bt_tiles = []
            batch_t = os.environ.get("V2_T", "1") == "1"
            for g in range(ngrp):
                if i_b > 0:
                    chunks = [(2 * g, 0), (2 * g, 1), (2 * g + 1, 0), (2 * g + 1, 1)]
                else:
                    chunks = [(4 * g + q, 1) for q in range(4)]
                bt = bt_p.tile([P, 512], bf16, tag="bt")
                if batch_t:
                    pt = ps_t.tile([P, 512], bf16, tag="pst")
                    for q, (h, half) in enumerate(chunks):
                        nc.tensor.matmul(
                            pt[:, q * P:(q + 1) * P],
                            bandu[:, h * 256 + half * P: h * 256 + (half + 1) * P],
                            identb[:],
                            is_transpose=True, start=(q == 0), stop=(q == 3),
                            skip_group_check=True,
                        )
                    if g % 2 == 0:
                        nc.scalar.copy(bt[:], pt[:])
                    else:
                        nc.vector.tensor_copy(bt[:], pt[:])
                else:
                    for q, (h, half) in enumerate(chunks):
                        pt = ps_t.tile([P, P], bf16, tag="pst")
                        nc.tensor.transpose(
                            pt[:],
                            bandu[:, h * 256 + half * P: h * 256 + (half + 1) * P],
                            identb[:],
                        )
                        if (g + q) % 2 == 0:
                            nc.scalar.copy(bt[:, q * P:(q + 1) * P], pt[:])
                        else:
                            nc.vector.tensor_copy(bt[:, q * P:(q + 1) * P], pt[:])
                bt_tiles.append(bt)

            def _band(h, half):
                # returns (tile, col0) of Band^T chunk for head h
                if i_b > 0:
                    return bt_tiles[h // 2], ((h % 2) * 2 + half) * P
                return bt_tiles[h // 4], (h % 4) * P

            # ---- conv matmuls: conv^T, 4 head-pairs per PSUM bank ----
            ct_tiles = []
            batch_c = os.environ.get("V2_C", "1") == "1"
            for g2 in range(2):
                pc = ps_c.tile([P, 512], f32, tag="psc")
                # start=True clears the pending-zero (has_written) state only
                # for the issuing matmul's partition range, so each 64-row
                # half needs its own group-opening matmul.
                started_hh = set()
                for hp_l in range(4):
                    hp = g2 * 4 + hp_l
                    cs = slice(hp_l * P, (hp_l + 1) * P)
                    for hh in range(2):
                        h = hp * 2 + hh
                        ms = slice(hh * 64, hh * 64 + 64)
                        first = (hh not in started_hh) if batch_c else True
                        started_hh.add(hh)
                        if not batch_c and i_b > 0:
                            first = True
                        if i_b > 0:
                            btt, c0 = _band(h, 0)
                            nc.tensor.matmul(
                                pc[ms, cs], h1_prev[:, h * R:(h + 1) * R],
                                btt[:, c0:c0 + P],
                                start=first, stop=False,
                                skip_group_check=True,
                            )
                            first = False
                        btt, c0 = _band(h, 1)
                        nc.tensor.matmul(
                            pc[ms, cs], h1_t[:, h * R:(h + 1) * R],
                            btt[:, c0:c0 + P],
                            start=first, stop=True,
                            skip_group_check=True,
                        )
                ct = ct_p.tile([P, 512], f32r, tag="ct")
                if g2 == 0:
                    nc.scalar.copy(ct[:], pc[:])
                else:
                    nc.vector.tensor_copy(ct[:], pc[:])
                ct_tiles.append(ct)

            # ---- Phase D: h2 (+b2) on PE; residual + stats on evac ----
            xtok_t = xtk_p.tile([P, E], f32, tag="xtok")
            nc.sync.dma_start(xtok_t[:], xtok_d[i * P:(i + 1) * P, :])
            zsb = z_p.tile([P, E], f32, tag="zsb")
            st = sm_p.tile([P, 8], f32, tag="st")
            sq = z_p.tile([P, E], f32, tag="sq")
            for eb in range(2):
                pd = ps_d.tile([P, 512], f32, tag="psd")
                for hp in range(8):
                    nc.tensor.matmul(
                        pd[:], ct_tiles[hp // 4][:, (hp % 4) * P:(hp % 4 + 1) * P],
                        w2T[hp][:, eb * 512:(eb + 1) * 512],
                        start=(hp == 0), stop=(hp == 7 and trivial_bias),
                    )
                if not trivial_bias:
                    nc.tensor.matmul(
                        pd[:], ones[:], b2r[:, eb * 512:(eb + 1) * 512],
                        start=False, stop=True,
                    )
                es = slice(eb * 512, (eb + 1) * 512)
                # z = h2 + x ; accum_out = sum(z)
                nc.vector.scalar_tensor_tensor(
                    zsb[:, es], pd[:], 0.0, xtok_t[:, es],
                    op0=ALU.add, op1=ALU.add, accum_out=st[:, eb:eb + 1],
                )
                # sum(z^2) via ACT Square (same table set)
                nc.scalar.activation(
                    sq[:, es], zsb[:, es], AF.Square,
                    accum_out=st[:, 4 + eb:5 + eb],
                )

            nc.vector.tensor_reduce(
                st[:, 2:3], st[:, 0:2], axis=mybir.AxisListType.X, op=ALU.add
            )
            nc.vector.tensor_scalar_mul(st[:, 3:4], st[:, 2:3], -1.0 / E)  # negmean
            nc.vector.tensor_reduce(
                st[:, 6:7], st[:, 4:6], axis=mybir.AxisListType.X, op=ALU.add
            )
            nc.vector.tensor_scalar(
                st[:, 7:8], st[:, 3:4], st[:, 3:4], None, op0=ALU.mult
            )  # m2 = negmean^2
            nc.vector.tensor_scalar(
                st[:, 6:7], st[:, 6:7], 1.0 / E, st[:, 7:8],
                op0=ALU.mult, op1=ALU.subtract,
            )  # var = sumsq/E - m2
            lnv = sm_p.tile([P, 2], f32, tag="lnv")
            nc.scalar.activation(lnv[:, 0:1], st[:, 6:7], AF.Ln, bias=eps_t[:, 0:1])
            nc.scalar.activation(lnv[:, 1:2], lnv[:, 0:1], AF.Exp, scale=-0.5)

            out_t = out_p.tile([P, E], f32, tag="outt")
            for eb in range(2):
                nc.vector.tensor_scalar(
                    out_t[:, eb * 512:(eb + 1) * 512],
                    zsb[:, eb * 512:(eb + 1) * 512],
                    st[:, 3:4], lnv[:, 1:2],
                    op0=ALU.add, op1=ALU.mult,
                )
            if not trivial_affine:
                nc.vector.tensor_mul(out_t[:], out_t[:], gam_t[:])
                nc.vector.tensor_add(out_t[:], out_t[:], bet_t[:])
            nc.sync.dma_start(out_d[i * P:(i + 1) * P, :], out_t[:])

            h1_prev = h1_t

    nc.finalize()
    return nc


def _scatter_idx() -> list[np.ndarray]:
    tables = []
    for h0, nh in SCAT_GROUPS:
        t = np.zeros((P, nh * KW), np.int16)
        for p in range(P):
            for hl in range(nh):
                for k in range(KW):
                    t[p, hl * KW + k] = hl * 256 + p + k + 98
        tables.append(t)
    return tables


_CACHE: dict = {}


def _get_nc(t_loc: int, trivial: bool, trivial_bias: bool = True):
    key = (t_loc, trivial, trivial_bias)
    if key not in _CACHE:
        _CACHE[key] = _build(t_loc, trivial, trivial_bias)
    return _CACHE[key]


def kernel(x, w1, b1, ww, bw, w2, b2, gamma, beta):
    x = np.asarray(x, np.float32)
    w1 = np.asarray(w1, np.float32)
    b1 = np.asarray(b1, np.float32)
    ww = np.asarray(ww, np.float32)
    bw = np.asarray(bw, np.float32)
    w2 = np.asarray(w2, np.float32)
    b2 = np.asarray(b2, np.float32)
    gamma = np.asarray(gamma, np.float32)
    beta = np.asarray(beta, np.float32)

    t_loc, b_full, e = x.shape
    assert e == E and b_full == B

    trivial = bool(np.all(gamma == 1.0) and np.all(beta == 0.0))
    wf = (ww.astype(np.float64) @ w1.astype(np.float64)).astype(np.float32)
    bwf = (ww.astype(np.float64) @ b1.astype(np.float64)).astype(np.float32) + bw
    trivial_bias = bool(
        np.all(b1 == 0.0) and np.all(bwf == 0.0) and np.all(b2 == 0.0)
    )
    nc = _get_nc(t_loc, trivial, trivial_bias)

    bf16 = mybir.dt.np(mybir.dt.bfloat16)
    common = {
        "w1T": np.ascontiguousarray(w1.T),
        "wfT": np.ascontiguousarray(wf.T),
        "w2T": np.ascontiguousarray(w2.T),
        "identb": np.eye(P).astype(bf16),
    }
    if not trivial_bias:
        common["b1r"] = b1[None, :]
        common["bwr"] = bwf[None, :]
        common["b2r"] = b2[None, :]
        common["ones"] = np.ones((1, P), np.float32)
    for g, t in enumerate(_scatter_idx()):
        common[f"idx{g}"] = t
    if not trivial:
        common["gamma_bc"] = np.broadcast_to(gamma, (P, E)).copy()
        common["beta_bc"] = np.broadcast_to(beta, (P, E)).copy()

    in_maps = []
    for c in range(NCORES):
        xs = x[:, NB * c:NB * (c + 1), :]
        xtok = np.ascontiguousarray(xs.transpose(1, 0, 2)).reshape(NB * t_loc, E)
        xT = np.ascontiguousarray(xs.transpose(2, 1, 0)).reshape(E, NB * t_loc)
        m = dict(common)
        m["xT"] = xT
        m["xtok"] = np.ascontiguousarray(xtok)
        in_maps.append(m)

    from concourse.bass_utils import run_bass_kernel_spmd

    res = run_bass_kernel_spmd(nc, in_maps, core_ids=list(range(NCORES)))

    out = np.empty((t_loc, B, E), np.float32)
    for c in range(NCORES):
        oc = res.results[c]["out"].reshape(NB, t_loc, E)
        for bl in range(NB):
            out[:, NB * c + bl, :] = oc[bl]
    return out


# revision 17
# speedup vs baseline: 32148.4472x; 29457.9646x over previous
"""Trainium2 Bass kernel for a DynamicConv decoder layer.

Computation (fairseq DynamicConvDecoderLayer, eval mode, normalize_after):
    h  = x @ w1.T + b1                       # [T,B,E] -> [T,B,C]
    w  = softmax((h @ ww.T + bw) per-head)   # dynamic conv weights [T,B,H,K]
    c  = causal banded aggregation of h with per-position weights
    h2 = c @ w2.T + b2
    out = LayerNorm(x + h2) * gamma + beta

Distribution: data-parallel over batch (B=16 -> 2 per core on 8 cores).

Per-core algorithm (tokens laid out b-major, m = b*T + t):
  - Phase A: h1 = x @ w1.T (token-partition layout) via fp32r matmuls,
    lhsT = x^T (host pre-transposed), rhs = w1^T.
  - Phase B: conv logits computed directly from x with the host-fused
    weight (ww @ w1)^T, so h1 is never needed in C-partition layout.
  - Softmax per (token, head) on DVE/ACT; result cast to bf16.
  - Band build: GPSIMD local_scatter skews the per-token weight rows into
    an aligned band block Band[tau_out, tau_src] (per head), then PE
    transposes 128x128 chunks (4 per PSUM bank) to Band^T[tau_src, tau_out].
  - Conv: per (head, tau_out tile) 2 accumulating bf16 matmuls:
    conv^T[r, tau_out] = sum_{tau_src} h1[tau_src, r] * Band^T[tau_src, tau_out],
    4 head-pairs packed per PSUM bank; output lands in C-partition layout.
  - Phase D: h2 = conv @ w2.T with lhsT = conv^T; residual + sum(z) ride the
    PSUM->SBUF evacuation (scalar_tensor_tensor with accum_out); sum(z^2)
    rides an ACT Square pass.
  - LayerNorm rstd = exp(-0.5*ln(var+eps)); all ACT functions (Exp, Ln,
    Copy, Square) live in the single `natural_log_exp_and_others` table set.
"""

import sys
import os

sys.path.insert(0, "/opt/trn_rl_repo")

import numpy as np
from contextlib import ExitStack

import concourse.bass as bass
import concourse.bacc as bacc
import concourse.mybir as mybir
from concourse import tile

T, B, E = 2048, 16, 1024
CDIM, H, KW = 1024, 16, 31
R = CDIM // H            # 64 channels per head
NB = 2                   # batch shard per core
NCORES = 8
P = 128
EPS = 1e-5

AF = mybir.ActivationFunctionType
ALU = mybir.AluOpType

# local_scatter groups: (head0, nheads); num_idxs = nh*31 must be even,
# num_elems = nh*256 must be < 2048.
SCAT_GROUPS = [(0, 6), (6, 6), (12, 4)]

_ONE_TABLE = "natural_log_exp_and_others"


class _Bacc(bacc.Bacc):
    """Bacc with the ACT table list restricted to one set covering every
    activation function this kernel uses (Exp, Ln, Copy, Square, Identity)
    — the default per-activation selection ping-pongs between sets,
    costing a ~1.3us table load per switch."""

    def insert_act_table_loads(self):
        from concourse.hw_specs import get_activation_tables
        import bass_rust as _bass_rust

        has_activation = any(
            isinstance(i, mybir.InstActivation)
            for b in self.main_func.blocks
            for i in b.instructions
        )
        if not has_activation:
            return
        # Keep every entry (act_func_set_id is positional into
        # act_info.json) but empty the other sets so the selector can
        # only ever pick _ONE_TABLE.
        tables = [
            (k, v if k == _ONE_TABLE else set())
            for k, v in get_activation_tables(self.m.arch).items()
        ]
        assert any(v for _, v in tables)
        import bass_rust
        bass_rust.insert_act_table_loads(self, tables)


def _build(t_loc: int, trivial_affine: bool, trivial_bias: bool) -> bacc.Bacc:
    f32 = mybir.dt.float32
    f32r = mybir.dt.float32r
    bf16 = mybir.dt.bfloat16
    i16 = mybir.dt.int16

    m_loc = NB * t_loc           # tokens per core
    nt = m_loc // P              # token tiles
    tpb = t_loc // P             # tiles per local batch
    nblk = max(m_loc // 512, 1)  # 512-token xT blocks
    tpblk = nt // nblk           # tiles per block (4)

    nc = _Bacc()

    xT_d = nc.dram_tensor("xT", [E, m_loc], f32r, kind="ExternalInput")
    xtok_d = nc.dram_tensor("xtok", [m_loc, E], f32, kind="ExternalInput")
    w1T_d = nc.dram_tensor("w1T", [E, CDIM], f32r, kind="ExternalInput")
    wfT_d = nc.dram_tensor("wfT", [E, H * KW], f32r, kind="ExternalInput")
    w2T_d = nc.dram_tensor("w2T", [CDIM, E], f32r, kind="ExternalInput")
    identb_d = nc.dram_tensor("identb", [P, P], bf16, kind="ExternalInput")
    idx_d = [
        nc.dram_tensor(f"idx{g}", [P, nh * KW], i16, kind="ExternalInput")
        for g, (_, nh) in enumerate(SCAT_GROUPS)
    ]
    if not trivial_bias:
        b1r_d = nc.dram_tensor("b1r", [1, CDIM], f32r, kind="ExternalInput")
        bwr_d = nc.dram_tensor("bwr", [1, H * KW], f32r, kind="ExternalInput")
        b2r_d = nc.dram_tensor("b2r", [1, E], f32r, kind="ExternalInput")
        ones_d = nc.dram_tensor("ones", [1, P], f32r, kind="ExternalInput")
    if not trivial_affine:
        gam_d = nc.dram_tensor("gamma_bc", [P, E], f32, kind="ExternalInput")
        bet_d = nc.dram_tensor("beta_bc", [P, E], f32, kind="ExternalInput")
    out_d = nc.dram_tensor("out", [m_loc, E], f32, kind="ExternalOutput")

    with tile.TileContext(nc) as tc, ExitStack() as ctx:
        const = ctx.enter_context(tc.tile_pool(name="const", bufs=1))
        xt_p = ctx.enter_context(tc.tile_pool(name="xt", bufs=2))
        xtk_p = ctx.enter_context(tc.tile_pool(name="xtk", bufs=2))
        h1_p = ctx.enter_context(tc.tile_pool(name="h1", bufs=4 if (trivial_affine and trivial_bias) else 3))
        sm_p = ctx.enter_context(tc.tile_pool(name="sm", bufs=2))
        bu_p = ctx.enter_context(tc.tile_pool(name="bu", bufs=2))
        bt_p = ctx.enter_context(tc.tile_pool(name="bt", bufs=12 if (trivial_affine and trivial_bias) else 8))
        ct_p = ctx.enter_context(tc.tile_pool(name="ct", bufs=3))
        z_p = ctx.enter_context(tc.tile_pool(name="z", bufs=2))
        out_p = ctx.enter_context(tc.tile_pool(name="outp", bufs=2))
        ps_ab = ctx.enter_context(tc.tile_pool(name="psab", bufs=3, space="PSUM"))
        ps_d = ctx.enter_context(tc.tile_pool(name="psd", bufs=2, space="PSUM"))
        ps_t = ctx.enter_context(tc.tile_pool(name="pst", bufs=1, space="PSUM"))
        ps_c = ctx.enter_context(tc.tile_pool(name="psc", bufs=2, space="PSUM"))

        # resident constants. DMA order matters at startup: the first
        # matmuls need xT block 0 and w1T/wfT; w2T is only needed ~10us in,
        # so it goes last to shorten the initial PE stall.
        w1T = [const.tile([P, CDIM], f32r, tag=f"w1T{e}", name=f"w1T{e}")
               for e in range(8)]
        wfT = [const.tile([P, H * KW], f32r, tag=f"wfT{e}", name=f"wfT{e}")
               for e in range(8)]
        w2T = [const.tile([P, E], f32r, tag=f"w2T{c}", name=f"w2T{c}")
               for c in range(8)]
        xt0 = [xt_p.tile([P, min(512, m_loc)], f32r, tag=f"xt{e}",
                         name=f"xtt0{e}") for e in range(8)]
        for e in range(8):
            nc.sync.dma_start(xt0[e][:], xT_d[e * P:(e + 1) * P, 0:min(512, m_loc)])
            nc.sync.dma_start(w1T[e][:], w1T_d[e * P:(e + 1) * P, :])
        for e in range(8):
            nc.sync.dma_start(wfT[e][:], wfT_d[e * P:(e + 1) * P, :])
        identb = const.tile([P, P], bf16, tag="identb")
        nc.sync.dma_start(identb[:], identb_d[:])
        for e in range(8):
            nc.sync.dma_start(w2T[e][:], w2T_d[e * P:(e + 1) * P, :])
        eps_t = const.tile([P, 1], f32, tag="eps")
        nc.vector.memset(eps_t[:], EPS)
        if not trivial_bias:
            b1r = const.tile([1, CDIM], f32r, tag="b1r")
            bwr = const.tile([1, H * KW], f32r, tag="bwr")
            b2r = const.tile([1, E], f32r, tag="b2r")
            ones = const.tile([1, P], f32r, tag="ones")
            nc.sync.dma_start(b1r[:], b1r_d[:])
            nc.sync.dma_start(bwr[:], bwr_d[:])
            nc.sync.dma_start(b2r[:], b2r_d[:])
            nc.sync.dma_start(ones[:], ones_d[:])
        idx_t = []
        for g, (_, nh) in enumerate(SCAT_GROUPS):
            it = const.tile([P, nh * KW], i16, tag=f"idx{g}", name=f"idxt{g}")
            nc.sync.dma_start(it[:], idx_d[g][:])
            idx_t.append(it)
        if not trivial_affine:
            gam_t = const.tile([P, E], f32, tag="gam")
            bet_t = const.tile([P, E], f32, tag="bet")
            nc.sync.dma_start(gam_t[:], gam_d[:])
            nc.sync.dma_start(bet_t[:], bet_d[:])

        xt = None
        h1_prev = None

        for i in range(nt):
            i_b = i % tpb
            j = i % tpblk
            if j == 0:
                blk = i // tpblk
                bw_ = min(512, m_loc)
                if blk == 0:
                    xt = xt0
                else:
                    xt = [xt_p.tile([P, bw_], f32r, tag=f"xt{e}", name=f"xtt{e}")
                          for e in range(8)]
                    for e in range(8):
                        nc.sync.dma_start(
                            xt[e][:],
                            xT_d[e * P:(e + 1) * P, blk * bw_:(blk + 1) * bw_]
                        )
            js = slice(j * P, (j + 1) * P)

            # ---- Phases A+B fused e-major: the three matmuls per E-chunk
            # share one stationary lhsT (the xT slice), so the PE reloads
            # weights once per chunk instead of three times. ----
            h1_t = h1_p.tile([P, CDIM], bf16, tag="h1")
            pa0 = ps_ab.tile([P, 512], f32, tag="psab")
            pa1 = ps_ab.tile([P, 512], f32, tag="psab")
            pb = ps_ab.tile([P, H * KW], f32, tag="psab")
            pas = [pa0, pa1]
            for e in range(8):
                last = e == 7 and trivial_bias
                nc.tensor.matmul(pa0[:], xt[e][:, js], w1T[e][:, 0:512],
                                 start=(e == 0), stop=last)
                nc.tensor.matmul(pa1[:], xt[e][:, js], w1T[e][:, 512:1024],
                                 start=(e == 0), stop=last)
                nc.tensor.matmul(pb[:], xt[e][:, js], wfT[e][:],
                                 start=(e == 0), stop=last)
            if not trivial_bias:
                nc.tensor.matmul(pa0[:], ones[:], b1r[:, 0:512],
                                 start=False, stop=True)
                nc.tensor.matmul(pa1[:], ones[:], b1r[:, 512:1024],
                                 start=False, stop=True)
                nc.tensor.matmul(pb[:], ones[:], bwr[:], start=False, stop=True)
            for cb in range(2):
                nc.scalar.copy(h1_t[:, cb * 512:(cb + 1) * 512], pas[cb][:])
            expw = sm_p.tile([P, H * KW], f32, tag="expw")
            nc.scalar.activation(expw[:], pb[:], AF.Exp)
            sums = sm_p.tile([P, H], f32, tag="sums")
            nc.vector.tensor_reduce(
                sums[:], expw[:].rearrange("p (h k) -> p h k", k=KW),
                axis=mybir.AxisListType.X, op=ALU.add,
            )
            rsum = sm_p.tile([P, H], f32, tag="rsum")
            nc.vector.reciprocal(rsum[:], sums[:])
            wbf = sm_p.tile([P, H * KW], bf16, tag="wbf")
            for h in range(H):
                nc.vector.tensor_scalar_mul(
                    wbf[:, h * KW:(h + 1) * KW],
                    expw[:, h * KW:(h + 1) * KW],
                    rsum[:, h:h + 1],
                )

            # ---- band build: scatter to Band[tau_out, (h, sigma)] ----
            bandu = bu_p.tile([P, H * 256], bf16, tag="bandu")
            for g, (h0, nh) in enumerate(SCAT_GROUPS):
                nc.gpsimd.local_scatter(
                    bandu[:, h0 * 256:(h0 + nh) * 256],
                    wbf[:, h0 * KW:(h0 + nh) * KW],
                    idx_t[g][:],
                    channels=P, num_elems=nh * 256, num_idxs=nh * KW,
                )

            # ---- PE transposes: Band^T[tau_src, tau_out], 4 chunks/bank ----
            # i_b>0: group g covers head pair (2g, 2g+1): [lo0|hi0|lo1|hi1]
            # i_b==0: group g covers heads 4g..4g+3: [hi|hi|hi|hi]
            ngrp = 8 if i_b > 0 else 4
            bt_tiles = []
            batch_t = True
            for g in range(ngrp):
                if i_b > 0:
                    chunks = [(2 * g, 0), (2 * g, 1), (2 * g + 1, 0), (2 * g + 1, 1)]
                else:
                    chunks = [(4 * g + q, 1) for q in range(4)]
                bt = bt_p.tile([P, 512], bf16, tag="bt")
                if batch_t:
                    pt = ps_t.tile([P, 512], bf16, tag="pst")
                    for q, (h, half) in enumerate(chunks):
                        nc.tensor.matmul(
                            pt[:, q * P:(q + 1) * P],
                            bandu[:, h * 256 + half * P: h * 256 + (half + 1) * P],
                            identb[:],
                            is_transpose=True, start=(q == 0), stop=(q == 3),
                            skip_group_check=True,
                        )
                    if g % 2 == 0:
                        nc.scalar.copy(bt[:], pt[:])
                    else:
                        nc.vector.tensor_copy(bt[:], pt[:])
                else:
                    for q, (h, half) in enumerate(chunks):
                        pt = ps_t.tile([P, P], bf16, tag="pst")
                        nc.tensor.transpose(
                            pt[:],
                            bandu[:, h * 256 + half * P: h * 256 + (half + 1) * P],
                            identb[:],
                        )
                        if (g + q) % 2 == 0:
                            nc.scalar.copy(bt[:, q * P:(q + 1) * P], pt[:])
                        else:
                            nc.vector.tensor_copy(bt[:, q * P:(q + 1) * P], pt[:])
                bt_tiles.append(bt)

            def _band(h, half):
                # returns (tile, col0) of Band^T chunk for head h
                if i_b > 0:
                    return bt_tiles[h // 2], ((h % 2) * 2 + half) * P
                return bt_tiles[h // 4], (h % 4) * P

            # ---- conv matmuls: conv^T, 4 head-pairs per PSUM bank ----
            ct_tiles = []
            batch_c = True
            for g2 in range(2):
                pc = ps_c.tile([P, 512], f32, tag="psc")
                # start=True clears the pending-zero (has_written) state only
                # for the issuing matmul's partition range, so each 64-row
                # half needs its own group-opening matmul.
                started_hh = set()
                for hp_l in range(4):
                    hp = g2 * 4 + hp_l
                    cs = slice(hp_l * P, (hp_l + 1) * P)
                    for hh in range(2):
                        h = hp * 2 + hh
                        ms = slice(hh * 64, hh * 64 + 64)
                        first = (hh not in started_hh) if batch_c else True
                        started_hh.add(hh)
                        if not batch_c and i_b > 0:
                            first = True
                        if i_b > 0:
                            btt, c0 = _band(h, 0)
                            nc.tensor.matmul(
                                pc[ms, cs], h1_prev[:, h * R:(h + 1) * R],
                                btt[:, c0:c0 + P],
                                start=first, stop=False,
                                skip_group_check=True,
                            )
                            first = False
                        btt, c0 = _band(h, 1)
                        nc.tensor.matmul(
                            pc[ms, cs], h1_t[:, h * R:(h + 1) * R],
                            btt[:, c0:c0 + P],
                            start=first, stop=True,
                            skip_group_check=True,
                        )
                ct = ct_p.tile([P, 512], f32r, tag="ct")
                if g2 == 0:
                    nc.scalar.copy(ct[:], pc[:])
                else:
                    nc.vector.tensor_copy(ct[:], pc[:])
                ct_tiles.append(ct)

            # ---- Phase D: h2 (+b2) on PE; residual + stats on evac ----
            xtok_t = xtk_p.tile([P, E], f32, tag="xtok")
            nc.sync.dma_start(xtok_t[:], xtok_d[i * P:(i + 1) * P, :])
            zsb = z_p.tile([P, E], f32, tag="zsb")
            st = sm_p.tile([P, 8], f32, tag="st")
            sq = z_p.tile([P, E], f32, tag="sq")
            pds = [ps_d.tile([P, 512], f32, tag="psd", name=f"pd{eb}")
                   for eb in range(2)]
            for hp in range(8):
                lhs = ct_tiles[hp // 4][:, (hp % 4) * P:(hp % 4 + 1) * P]
                for eb in range(2):
                    nc.tensor.matmul(
                        pds[eb][:], lhs,
                        w2T[hp][:, eb * 512:(eb + 1) * 512],
                        start=(hp == 0), stop=(hp == 7 and trivial_bias),
                    )
            if not trivial_bias:
                for eb in range(2):
                    nc.tensor.matmul(
                        pds[eb][:], ones[:], b2r[:, eb * 512:(eb + 1) * 512],
                        start=False, stop=True,
                    )
            for eb in range(2):
                es = slice(eb * 512, (eb + 1) * 512)
                # z = h2 + x ; accum_out = sum(z)
                nc.vector.scalar_tensor_tensor(
                    zsb[:, es], pds[eb][:], 0.0, xtok_t[:, es],
                    op0=ALU.add, op1=ALU.add, accum_out=st[:, eb:eb + 1],
                )
                # sum(z^2) via ACT Square (same table set)
                nc.scalar.activation(
                    sq[:, es], zsb[:, es], AF.Square,
                    accum_out=st[:, 4 + eb:5 + eb],
                )

            nc.vector.tensor_reduce(
                st[:, 2:3], st[:, 0:2], axis=mybir.AxisListType.X, op=ALU.add
            )
            nc.vector.tensor_scalar_mul(st[:, 3:4], st[:, 2:3], -1.0 / E)  # negmean
            nc.vector.tensor_reduce(
                st[:, 6:7], st[:, 4:6], axis=mybir.AxisListType.X, op=ALU.add
            )
            nc.vector.tensor_scalar(
                st[:, 7:8], st[:, 3:4], st[:, 3:4], None, op0=ALU.mult
            )  # m2 = negmean^2
            nc.vector.tensor_scalar(
                st[:, 6:7], st[:, 6:7], 1.0 / E, st[:, 7:8],
                op0=ALU.mult, op1=ALU.subtract,
            )  # var = sumsq/E - m2
            lnv = sm_p.tile([P, 2], f32, tag="lnv")
            nc.scalar.activation(lnv[:, 0:1], st[:, 6:7], AF.Ln, bias=eps_t[:, 0:1])
            nc.scalar.activation(lnv[:, 1:2], lnv[:, 0:1], AF.Exp, scale=-0.5)

            out_t = out_p.tile([P, E], f32, tag="outt")
            for eb in range(2):
                nc.vector.tensor_scalar(
                    out_t[:, eb * 512:(eb + 1) * 512],
                    zsb[:, eb * 512:(eb + 1) * 512],
                    st[:, 3:4], lnv[:, 1:2],
                    op0=ALU.add, op1=ALU.mult,
                )
            if not trivial_affine:
                nc.vector.tensor_mul(out_t[:], out_t[:], gam_t[:])
                nc.vector.tensor_add(out_t[:], out_t[:], bet_t[:])
            nc.sync.dma_start(out_d[i * P:(i + 1) * P, :], out_t[:])

            h1_prev = h1_t

    nc.finalize()
    return nc


def _scatter_idx() -> list[np.ndarray]:
    tables = []
    for h0, nh in SCAT_GROUPS:
        t = np.zeros((P, nh * KW), np.int16)
        for p in range(P):
            for hl in range(nh):
                for k in range(KW):
                    t[p, hl * KW + k] = hl * 256 + p + k + 98
        tables.append(t)
    return tables


_CACHE: dict = {}


def _get_nc(t_loc: int, trivial: bool, trivial_bias: bool = True):
    key = (t_loc, trivial, trivial_bias)
    if key not in _CACHE:
        _CACHE[key] = _build(t_loc, trivial, trivial_bias)
    return _CACHE[key]


def kernel(x, w1, b1, ww, bw, w2, b2, gamma, beta):
    x = np.asarray(x, np.float32)
    w1 = np.asarray(w1, np.float32)
    b1 = np.asarray(b1, np.float32)
    ww = np.asarray(ww, np.float32)
    bw = np.asarray(bw, np.float32)
    w2 = np.asarray(w2, np.float32)
    b2 = np.asarray(b2, np.float32)
    gamma = np.asarray(gamma, np.float32)
    beta = np.asarray(beta, np.float32)

    t_loc, b_full, e = x.shape
    assert e == E and b_full == B

    trivial = bool(np.all(gamma == 1.0) and np.all(beta == 0.0))
    wf = (ww.astype(np.float64) @ w1.astype(np.float64)).astype(np.float32)
    bwf = (ww.astype(np.float64) @ b1.astype(np.float64)).astype(np.float32) + bw
    trivial_bias = bool(
        np.all(b1 == 0.0) and np.all(bwf == 0.0) and np.all(b2 == 0.0)
    )
    nc = _get_nc(t_loc, trivial, trivial_bias)

    bf16 = mybir.dt.np(mybir.dt.bfloat16)
    common = {
        "w1T": np.ascontiguousarray(w1.T),
        "wfT": np.ascontiguousarray(wf.T),
        "w2T": np.ascontiguousarray(w2.T),
        "identb": np.eye(P).astype(bf16),
    }
    if not trivial_bias:
        common["b1r"] = b1[None, :]
        common["bwr"] = bwf[None, :]
        common["b2r"] = b2[None, :]
        common["ones"] = np.ones((1, P), np.float32)
    for g, t in enumerate(_scatter_idx()):
        common[f"idx{g}"] = t
    if not trivial:
        common["gamma_bc"] = np.broadcast_to(gamma, (P, E)).copy()
        common["beta_bc"] = np.broadcast_to(beta, (P, E)).copy()

    in_maps = []
    for c in range(NCORES):
        xs = x[:, NB * c:NB * (c + 1), :]
        xtok = np.ascontiguousarray(xs.transpose(1, 0, 2)).reshape(NB * t_loc, E)
        xT = np.ascontiguousarray(xs.transpose(2, 1, 0)).reshape(E, NB * t_loc)
        m = dict(common)
        m["xT"] = xT
        m["xtok"] = np.ascontiguousarray(xtok)
        in_maps.append(m)

    from concourse.bass_utils import run_bass_kernel_spmd

    res = run_bass_kernel_spmd(nc, in_maps, core_ids=list(range(NCORES)))

    out = np.empty((t_loc, B, E), np.float32)
    for c in range(NCORES):
        oc = res.results[c]["out"].reshape(NB, t_loc, E)
        for bl in range(NB):
            out[:, NB * c + bl, :] = oc[bl]
    return out
